# revision 56
# baseline (speedup 1.0000x reference)
"""GQA attention layer (B=2, S=2048, D=4096, 32 Q heads / 8 KV heads, RoPE,
causal) on 8 Trainium2 NeuronCores, tensor-parallel over heads.

Each core owns 4 Q heads + 1 KV head: it computes its Q/K/V projections,
RoPE, causal attention, and a partial output projection (rank-512 slice of
the wo contraction).  The host sums the 8 partial outputs.

v2 design vs the fp32r baseline:
 - all matmul operands in bf16 (PSUM accumulation stays fp32): the PE
   streams bf16 at ~2x the fp32r rate and every DMA byte count halves.
 - q/k/v stay resident in SBUF (no DRAM round-trip between projection and
   attention), with V transposed to token-major via DMA-transpose.
 - exact-causal subblocking: the diagonal 512-block only computes the
   valid q range per 128-wide key chunk (saves ~15% of attention work).
 - attention inner loop is software-pipelined: the scores matmul of chunk
   i+1 issues before the AV matmul of chunk i, so the PE never waits on
   the Exp activation.
 - PSUM evictions are spread across ACT and DVE; y eviction on DVE.
 - weights load per-k-chunk so the first projection matmul starts ~10us
   after launch instead of waiting for the full weight load.
"""

import os
import sys
import types
from contextlib import ExitStack

import numpy as np
import ml_dtypes

import concourse.bass as bass
import concourse.tile as tile
from concourse import bacc
from concourse import mybir
from concourse import bass_utils
from concourse.bass_utils import run_bass_kernel_spmd

# ---------------------------------------------------------------------------
# Optional NTFF profiling support under axon. The trimmed image's `antenv`
# lacks `axon_hooks`, so run_bass_kernel_spmd(trace=True) would silently skip
# tracing; register the hook ourselves. Harmless when unavailable.
try:
    import antenv  # noqa: F401
    from trn_agent_boot.trn_boot import _ntff_profile_via_ctypes

    if "antenv.axon_hooks" not in sys.modules:
        _hooks_mod = types.ModuleType("antenv.axon_hooks")
        _hook = _ntff_profile_via_ctypes("/opt/axon/libaxon_pjrt.so")
        _hooks_mod.get_axon_ntff_profile_hook = lambda: _hook
        _hooks_mod.set_axon_ntff_profile_hook = lambda h: None
        sys.modules["antenv.axon_hooks"] = _hooks_mod
    bass_utils.upload_artifacts = lambda tmpdir: "local://skipped"
except Exception:
    pass

F32 = mybir.dt.float32
BF = mybir.dt.bfloat16
EXP = mybir.ActivationFunctionType.Exp

B, S, D = 2, 2048, 4096
NH, NKV, HD = 32, 8, 128
T = B * S                       # 4096 tokens total
N_CORES = 8
QH = NH // N_CORES              # 4 local q heads
FL = QH * HD                    # 512 local q features
SCALE = 1.0 / float(np.sqrt(HD))
NEG = -1.0e30

NW = 512                        # token-group width in the QKV projection
QB = 512                        # q-block width in attention
DKD = D // 128                  # 32 contraction chunks for projections
NG = T // NW                    # 8 token groups
GPB = S // NW                   # 4 groups per batch


def _build_program():
    nc = bacc.Bacc("TRN2", target_bir_lowering=False, debug=False,
                   num_devices=N_CORES)

    xT = nc.dram_tensor("xT", [D, T], BF, kind="ExternalInput").ap()
    wqT = nc.dram_tensor("wqT", [D, FL], BF, kind="ExternalInput").ap()
    wkT = nc.dram_tensor("wkT", [D, HD], BF, kind="ExternalInput").ap()
    wvT = nc.dram_tensor("wvT", [D, HD], BF, kind="ExternalInput").ap()
    woT = nc.dram_tensor("woT", [FL, D], BF, kind="ExternalInput").ap()
    # RoPE constants, pre-assembled for the rotate-half formulation on the
    # even/odd-split feature layout: ropc = [cos; cos], rops = [-sin; sin].
    ropc = nc.dram_tensor("ropc", [HD, S], F32, kind="ExternalInput").ap()
    rops = nc.dram_tensor("rops", [HD, S], F32, kind="ExternalInput").ap()
    onesin = nc.dram_tensor("onesin", [128, 1], BF, kind="ExternalInput").ap()
    # 128x128 strictly-lower-triangular causal mask (rows = key offset,
    # cols = query offset within the diagonal chunk): 0 where q >= k.
    masktri = nc.dram_tensor("masktri", [128, 128], F32,
                             kind="ExternalInput").ap()
    y = nc.dram_tensor("y", [T, D], F32, kind="ExternalOutput").ap()

    with tile.TileContext(nc) as tc, ExitStack() as ctx:
        const = ctx.enter_context(tc.tile_pool(name="const", bufs=1))
        resident = ctx.enter_context(tc.tile_pool(name="resident", bufs=1))

        # Resident bf16 activations (per batch / head): feature-major q/k,
        # token-major V.
        q_sb = [resident.tile([128, S], BF, tag=f"q_sb{i}", name=f"q_sb{i}")
                for i in range(B * QH)]
        k_sb = [resident.tile([128, S], BF, tag=f"k_sb{b}", name=f"k_sb{b}")
                for b in range(B)]
        V_b = [resident.tile([128, S], BF, tag=f"V_b{b}", name=f"V_b{b}")
               for b in range(B)]
        wo_sb = resident.tile([128, QH * D], BF)

        ones_t = const.tile([128, 1], BF)
        nc.gpsimd.dma_start(ones_t[:], onesin)
        mtri = const.tile([128, 128], F32)
        nc.gpsimd.dma_start(mtri[:], masktri)
        cos_s = const.tile([HD, S], F32)
        sin_s = const.tile([HD, S], F32)

        # ------------------------------------------------------------------
        # Phase 1: QKV projections + RoPE -> resident SBUF (feature-major)
        #
        # DMA streams are split by issuing engine so nothing head-of-line
        # blocks the x loads that feed the PE:
        #   sync   — x tile loads + weight loads (WAR-paced, stays ~6 tiles
        #            ahead of the PE), wo prefetch in small slabs
        #   scalar — PSUM evict copies (its compute), V transposes
        #   gpsimd — rope swap-halves, cos/sin
        # The SBUF pools stay open through phase 2 (outer ctx) so the
        # attention pools get fresh addresses and never WAR-wait on the
        # rope tail of the last group.
        # ------------------------------------------------------------------
        rtmp = ctx.enter_context(tc.tile_pool(name="ropetmp", bufs=2))
        xsbp = ctx.enter_context(tc.tile_pool(name="xsbp", bufs=7))
        xswp = ctx.enter_context(tc.tile_pool(name="xswp", bufs=6))
        vstage = ctx.enter_context(tc.tile_pool(name="vstage", bufs=2))
        with tc.tile_pool(name="wqkv", bufs=1) as wpool, \
             tc.tile_pool(name="xin", bufs=10) as xpool, \
             tc.tile_pool(name="qkvpsA", bufs=2, space="PSUM") as qkvpsA, \
             tc.tile_pool(name="qkvpsB", bufs=1, space="PSUM") as qkvpsB:

            # Resident weights, packed k-chunk-major, loaded in small slabs
            # so the first projection matmuls start almost immediately.
            wq_sb = wpool.tile([128, DKD * FL], BF, tag="wq")
            wk_sb = wpool.tile([128, DKD * HD], BF, tag="wk")
            wv_sb = wpool.tile([128, DKD * HD], BF, tag="wv")
            KS = DKD // 8

            def load_w_slab(w_sb, wT, width, p, nslab, eng=None):
                ks = DKD // nslab
                (eng or nc.sync).dma_start(
                    w_sb[:, p * ks * width:(p + 1) * ks * width]
                        .rearrange("p (k f) -> p k f", k=ks),
                    wT[p * ks * 128:(p + 1) * ks * 128, :]
                        .rearrange("(k p) f -> p k f", p=128))

            nc.gpsimd.dma_start(cos_s[:], ropc)
            nc.gpsimd.dma_start(sin_s[:], rops)
            # Warm the gpsimd custom-op library (partition_broadcast loads a
            # ucode lib on first use — ~7us, better spent during startup).
            warm = const.tile([128, 1], BF)
            nc.gpsimd.partition_broadcast(warm[:], ones_t[0:1, :])

            def rope_copy(ps, use_dve):
                """PSUM -> SBUF eviction (frees the PSUM bank)."""
                xsb = xsbp.tile([128, NW], F32, tag="xsb", name="xsb")
                if use_dve:
                    nc.vector.tensor_copy(xsb[:], ps[:])
                else:
                    nc.scalar.copy(xsb[:], ps[:])
                return xsb

            def rope_swaps(xsb):
                """Issue the swap-halves DMAs (gpsimd); returns the swapped
                tile. Split from the muls so deferred tails can pre-issue
                swaps on the idle transition gpsimd and the later DVE muls
                never head-of-line wait on them."""
                xsw = xswp.tile([128, NW], F32, tag="xsw", name="xsw")
                nc.gpsimd.dma_start(xsw[0:64, :], xsb[64:128, :])
                nc.gpsimd.dma_start(xsw[64:128, :], xsb[0:64, :])
                return xsw

            def rope_muls(xsb, xsw, out_slice, pos0):
                """out = RoPE(xsb) on the even/odd-split feature layout:
                out = x * [c;c] + swap_halves(x) * [-s;s], bf16 out."""
                c = cos_s[:, pos0:pos0 + NW]
                s = sin_s[:, pos0:pos0 + NW]
                t1 = rtmp.tile([128, NW], F32, tag="t1")
                nc.vector.tensor_mul(t1[:], xsw[:], s)
                t2 = rtmp.tile([128, NW], F32, tag="t2")
                nc.vector.tensor_mul(t2[:], xsb[:], c)
                nc.vector.tensor_add(out_slice, t2[:], t1[:])

            def rope_tail(xsb, out_slice, pos0):
                rope_muls(xsb, rope_swaps(xsb), out_slice, pos0)

            XPRE = 6            # x tiles prefetched across the group boundary

            def xt_load(n, k):
                xt = xpool.tile([128, NW], BF, tag="xt", name="xt")
                nc.sync.dma_start(
                    xt[:], xT[k * 128:(k + 1) * 128, n * NW:(n + 1) * NW])
                return xt

            # First x tiles go out ahead of the weight burst so the PE can
            # start at ~10us; weight slabs are interleaved with group 0's
            # x loads in consumption order (slab p covers k = 4p..4p+3).
            # Startup: the first k-chunks of all three weights, then the
            # first x tile, so the PE starts at ~10us; remaining slabs are
            # interleaved with group 0's x loads in consumption order.
            load_w_slab(wq_sb, wqT, FL, 0, 16)
            load_w_slab(wk_sb, wkT, HD, 0, 16)
            load_w_slab(wv_sb, wvT, HD, 0, 16)
            xt_pre = [xt_load(0, 0)]
            load_w_slab(wq_sb, wqT, FL, 1, 16)
            load_w_slab(wk_sb, wkT, HD, 1, 16)
            load_w_slab(wv_sb, wvT, HD, 1, 16)
            xt_pre += [xt_load(0, 1), xt_load(0, 2)]
            load_w_slab(wq_sb, wqT, FL, 1, 8)
            load_w_slab(wk_sb, wkT, HD, 1, 8)
            load_w_slab(wv_sb, wvT, HD, 1, 8)
            xt_pre += [xt_load(0, k) for k in range(3, XPRE)]
            deferred_tails = []

            for n in range(NG):
                b = n // GPB
                pos0 = (n * NW) % S
                # qps0/qps1 are double-buffered (pool A) so the next group's
                # first matmuls never wait on this group's evictions.
                qps = [qkvpsA.tile([128, NW], F32, tag=f"qps{m}", name=f"qps{m}")
                       for m in range(2)]
                qps += [qkvpsB.tile([128, NW], F32, tag=f"qps{m}", name=f"qps{m}")
                        for m in range(2, QH)]
                kps = qkvpsB.tile([128, NW], F32, tag="kps")
                vps = qkvpsB.tile([128, NW], F32, tag="vps")
                for k in range(DKD):
                    if n == 0 and k % 4 == 2 and k < 26:
                        p = k // 4 + 2
                        load_w_slab(wq_sb, wqT, FL, p, 8)
                        load_w_slab(wk_sb, wkT, HD, p, 8)
                        load_w_slab(wv_sb, wvT, HD, p, 8)
                    xt = xt_pre[k] if k < XPRE else xt_load(n, k)
                    st = (k == 0)
                    sp = (k == DKD - 1)
                    for m in range(QH):
                        nc.tensor.matmul(
                            qps[m][:],
                            wq_sb[:, k * FL + m * 128:k * FL + (m + 1) * 128],
                            xt[:], start=st, stop=sp)
                    nc.tensor.matmul(
                        kps[:], wk_sb[:, k * HD:(k + 1) * HD], xt[:],
                        start=st, stop=sp)
                    nc.tensor.matmul(
                        vps[:], wv_sb[:, k * HD:(k + 1) * HD], xt[:],
                        start=st, stop=sp)
                # Prefetch next group's first x tiles (WAR on this group's
                # tail tiles resolves as the k-loop matmuls retire).
                if n + 1 < NG:
                    xt_pre = [xt_load(n + 1, k) for k in range(XPRE)]
                # Output-projection weights prefetch, in half-slabs small
                # enough not to starve the x-load stream.
                if 2 <= n < 6:
                    f = n - 2
                    nc.sync.dma_start(wo_sb[:, f * D:f * D + D // 2],
                                      woT[f * 128:(f + 1) * 128, :D // 2])
                    nc.sync.dma_start(wo_sb[:, f * D + D // 2:(f + 1) * D],
                                      woT[f * 128:(f + 1) * 128, D // 2:])
                # Copy phase: free all six PSUM tiles ASAP, single-buffered
                # ones first, alternating ACT/DVE; tails follow.
                x2 = rope_copy(qps[2], use_dve=False)
                x3 = rope_copy(qps[3], use_dve=True)
                xk = rope_copy(kps, use_dve=False)
                vst = vstage.tile([128, NW], BF, tag="vst")
                nc.vector.tensor_copy(vst[:], vps[:])
                x0 = rope_copy(qps[0], use_dve=False)
                x1 = rope_copy(qps[1], use_dve=True)
                # last group's transposes go via sync (idle by then) so they
                # don't head-of-line block phase 2's first Exp on ACT
                teng = nc.sync if n == NG - 1 else nc.scalar
                for j in range(NW // 128):
                    teng.dma_start_transpose(
                        V_b[b][:, pos0 + j * 128:pos0 + (j + 1) * 128],
                        vst[:, j * 128:(j + 1) * 128])
                tails = [(x2, q_sb[b * QH + 2][:, pos0:pos0 + NW]),
                         (x3, q_sb[b * QH + 3][:, pos0:pos0 + NW]),
                         (xk, k_sb[b][:, pos0:pos0 + NW]),
                         (x0, q_sb[b * QH + 0][:, pos0:pos0 + NW]),
                         (x1, q_sb[b * QH + 1][:, pos0:pos0 + NW])]
                if n < NG - 1:
                    for xs, sl in tails:
                        rope_tail(xs, sl, pos0)
                else:
                    # The last group's tails (batch-1 data, not needed until
                    # the b=1 attention) are deferred into phase 2 so they
                    # never head-of-line block the first mask-adds on DVE;
                    # their swaps run now, on the idle transition gpsimd.
                    deferred_tails = [(xs, rope_swaps(xs), sl, pos0)
                                      for xs, sl in tails]

        # ------------------------------------------------------------------
        # Phase 2: attention + output projection
        # ------------------------------------------------------------------
        with tc.tile_pool(name="attn", bufs=2) as atpool, \
             tc.tile_pool(name="smax", bufs=2) as smpool, \
             tc.tile_pool(name="ptiles", bufs=4) as ptpool, \
             tc.tile_pool(name="ystage", bufs=6) as ypool, \
             tc.tile_pool(name="sps", bufs=3, space="PSUM") as spsum, \
             tc.tile_pool(name="avps", bufs=2, space="PSUM") as avpsum, \
             tc.tile_pool(name="sums", bufs=1, space="PSUM") as smpsum, \
             tc.tile_pool(name="yps", bufs=2, space="PSUM") as ypsum:

            def _emit_av(avp, smp, vtb, pt, ktc, qoff, qw, first, last=False):
                nc.tensor.matmul(
                    avp[:, qoff:qoff + qw],
                    vtb[:, ktc * 128:(ktc + 1) * 128],
                    pt[:, qoff:qoff + qw], start=first, stop=last,
                    skip_group_check=True)
                nc.tensor.matmul(
                    smp[:, qoff:qoff + qw], ones_t[:], pt[:, qoff:qoff + qw],
                    start=first, stop=last, skip_group_check=True)

            def wo_blocks(att_prev, q0_prev, t0, last=False):
                """One zero-arg closure per (tcx, dg) output block, so the
                caller can interleave them into the attention chunk stream.
                Interleaved blocks evict on DVE only — ACT is the saturated
                exp path while attention runs; the final drain splits."""
                def mk(tcx, dg):
                    def emit():
                        tg0 = t0 + q0_prev + tcx * 128
                        yp = ypsum.tile([128, NW], F32)
                        for f in range(QH):
                            nc.tensor.matmul(
                                yp[:],
                                att_prev[f][:, tcx * 128:(tcx + 1) * 128],
                                wo_sb[:, f * D + dg * NW:f * D + (dg + 1) * NW],
                                start=(f == 0), stop=(f == QH - 1))
                        ysb = ypool.tile([128, NW], F32)
                        if last and dg % 2 == 1:
                            nc.scalar.copy(ysb[:], yp[:])
                        else:
                            nc.vector.tensor_copy(ysb[:], yp[:])
                        if last:
                            eng = (nc.gpsimd, nc.sync, nc.scalar)[dg % 3]
                        else:
                            eng = nc.gpsimd if dg % 2 == 0 else nc.sync
                        eng.dma_start(
                            y[tg0:tg0 + 128, dg * NW:(dg + 1) * NW], ysb[:])
                    return emit
                return [mk(tcx, dg) for tcx in range(QB // 128)
                        for dg in range(D // NW)]

            pending = None
            for b in range(B):
                t0 = b * S
                ktb = k_sb[b]
                vtb = V_b[b]
                # largest q-block first: the deep chunk pipeline at phase-2
                # entry hides Exp latency where there is no wo work yet, and
                # the ACT-bound short blocks land where wo emissions overlap
                for qbi, qb in enumerate(reversed(range(S // QB))):
                    q0 = qb * QB
                    # chunk list: full-width chunks below the diagonal block,
                    # then the 4 diagonal chunks with narrowing q ranges.
                    chunks = [(ktc, 0, QB, False) for ktc in range(qb * 4)]
                    chunks += [(qb * 4 + j, j * 128, QB - j * 128, True)
                               for j in range(4)]
                    att = [atpool.tile([128, QB], BF, tag=f"att{h}",
                                       name=f"att{h}") for h in range(QH)]
                    # One flat software pipeline across all 4 heads: stp/exp
                    # of chunk i issue before AV/sums of chunk i-1, and a
                    # head's normalize is emitted as soon as its last AV is,
                    # so the PE stream never waits on the ACT Exp and head
                    # boundaries cost nothing.
                    avp = {}
                    smp = {}

                    def flush(p):
                        h, pt, ktc, qoff, qw, first, last = p
                        _emit_av(avp[h], smp[h], vtb, pt, ktc, qoff, qw,
                                 first, last)
                        if last:
                            # normalize: att = avp * (1 / colsum) off PSUM.
                            # Colsum eviction engine: ACT is the critical exp
                            # path in short q-blocks (use DVE), DVE is the
                            # critical mask-add/WAR path in deep ones (ACT).
                            s_sb = smpool.tile([1, QB], F32, tag="s_sb")
                            if qb <= 1:
                                nc.vector.tensor_copy(s_sb[:], smp[h][:])
                            else:
                                nc.scalar.copy(s_sb[:], smp[h][:])
                            s_bc = smpool.tile([128, QB], F32, tag="s_bc")
                            nc.gpsimd.partition_broadcast(s_bc[:], s_sb[:])
                            r_bc = smpool.tile([128, QB], F32, tag="r_bc")
                            nc.vector.reciprocal_approx_fast(r_bc[:], s_bc[:])
                            nc.vector.tensor_mul(att[h][:], avp[h][:],
                                                 r_bc[:])

                    # depth-2 pipeline: AV/sums of chunk i-2 issue after
                    # stp/exp of chunk i, giving each Exp ~2 chunks of PE
                    # time to complete before its AV needs it.  The pending
                    # wo blocks interleave into the chunk stream so the PE
                    # has cover work wherever the ACT exp path is the limit.
                    blocks = wo_blocks(*pending) if pending is not None else []
                    bi = 0
                    nch = len(chunks) * QH
                    ci = 0
                    pipe = []
                    for h in range(QH):
                        qtb = q_sb[b * QH + h]
                        for i, (ktc, qoff, qw, diag) in enumerate(chunks):
                            if i == 0:
                                avp[h] = avpsum.tile([128, QB], F32,
                                                     tag="avp", name="avp")
                                smp[h] = smpsum.tile([1, QB], F32,
                                                     tag="smp", name="smp")
                            stp = spsum.tile([128, QB], F32, tag="stp")
                            nc.tensor.matmul(
                                stp[:, qoff:qoff + qw],
                                ktb[:, ktc * 128:(ktc + 1) * 128],
                                qtb[:, q0 + qoff:q0 + qoff + qw],
                                start=True, stop=True)
                            if diag:
                                nc.vector.tensor_add(
                                    stp[:, qoff:qoff + 128],
                                    stp[:, qoff:qoff + 128], mtri[:])
                            pt = ptpool.tile([128, QB], BF, tag="pt")
                            nc.scalar.activation(
                                pt[:, qoff:qoff + qw], stp[:, qoff:qoff + qw],
                                EXP, scale=SCALE)
                            pipe.append((h, pt, ktc, qoff, qw, i == 0,
                                         i == len(chunks) - 1))
                            if len(pipe) > 2:
                                flush(pipe.pop(0))
                            ci += 1
                            want = len(blocks) * ci // nch
                            while bi < want:
                                blocks[bi]()
                                bi += 1
                    for p in pipe:
                        flush(p)
                    while bi < len(blocks):
                        blocks[bi]()
                        bi += 1
                    # drip the deferred batch-1 rope tails into the DVE
                    # slack of batch 0's attention blocks, one per block so
                    # their swap->mul chains never back up the DVE queue
                    if b == 0 and deferred_tails:
                        take = 1 if qbi < 3 else len(deferred_tails)
                        for xs, xw, sl, p0 in deferred_tails[:take]:
                            rope_muls(xs, xw, sl, p0)
                        deferred_tails = deferred_tails[take:]
                    pending = (att, q0, t0)
            if pending is not None:
                for blk in wo_blocks(*pending, last=True):
                    blk()

    nc.compile()
    return nc


_program = None


def _get_program():
    global _program
    if _program is None:
        _program = _build_program()
    return _program


def kernel(**inputs) -> np.ndarray:
    x = np.asarray(inputs["x"], dtype=np.float32)
    wq = np.asarray(inputs["wq"], dtype=np.float32)
    wk = np.asarray(inputs["wk"], dtype=np.float32)
    wv = np.asarray(inputs["wv"], dtype=np.float32)
    wo = np.asarray(inputs["wo"], dtype=np.float32)
    cos = np.asarray(inputs["freqs_cos"], dtype=np.float32)
    sin = np.asarray(inputs["freqs_sin"], dtype=np.float32)
    start_pos = int(np.asarray(inputs.get("start_pos", 0)))
    assert start_pos == 0, "kernel specialized for start_pos == 0"

    BF_NP = ml_dtypes.bfloat16

    # Even/odd RoPE pair split within each head's 128 features.
    perm = np.concatenate([np.arange(0, HD, 2), np.arange(1, HD, 2)])

    xT = np.ascontiguousarray(x.reshape(T, D).T.astype(BF_NP))
    cosT = cos.T                                   # [64, S]
    sinT = sin.T
    ropc = np.ascontiguousarray(np.concatenate([cosT, cosT], axis=0))
    rops = np.ascontiguousarray(np.concatenate([-sinT, sinT], axis=0))
    # 128x128 diagonal-chunk mask: rows = key offset r, cols = query offset
    # c within the chunk; allowed (0.0) where c >= r.
    r = np.arange(128)
    masktri = np.where(r[None, :] >= r[:, None], 0.0, NEG).astype(np.float32)

    in_maps = []
    for c in range(N_CORES):
        wq_c = wq[c * FL:(c + 1) * FL].reshape(QH, HD, D)[:, perm, :].reshape(FL, D)
        wk_c = wk[c * HD:(c + 1) * HD][perm, :]
        wv_c = wv[c * HD:(c + 1) * HD]
        wo_c = wo[:, c * FL:(c + 1) * FL]
        in_maps.append({
            "xT": xT,
            "wqT": np.ascontiguousarray(wq_c.T.astype(BF_NP)),
            "wkT": np.ascontiguousarray(wk_c.T.astype(BF_NP)),
            "wvT": np.ascontiguousarray(wv_c.T.astype(BF_NP)),
            "woT": np.ascontiguousarray(wo_c.T.astype(BF_NP)),
            "ropc": ropc,
            "rops": rops,
            "onesin": np.ones((128, 1), dtype=BF_NP),
            "masktri": masktri,
        })

    nc = _get_program()
    trace = bool(int(os.environ.get("GQA_TRACE", "0")))
    kwargs = {}
    if trace:
        tmpdir = os.environ.get("GQA_TRACE_DIR") or None
        kwargs = dict(trace=True, tmpdir=tmpdir, trace_cores=[0])
    res = run_bass_kernel_spmd(nc, in_maps, list(range(N_CORES)), **kwargs)
    kernel.last_results = res

    acc = np.zeros((T, D), dtype=np.float64)
    for c in range(N_CORES):
        acc += res.results[c]["y"].astype(np.float32)
    return acc.astype(np.float32).reshape(B, S, D)
